# revision 29
# baseline (speedup 1.0000x reference)
# Dense GAT layer (4 heads, dim 64) on Trainium2 via Bass/Tile.
#
# Math: h = x@W; e_ij = LeakyReLU(src_i + dst_j, 0.2); masked softmax over j
# with valid = adj & mask_i & mask_j; out = LN((alpha @ h) * mask_i).
#
# Host-side compaction: masked (mask=0) nodes contribute exactly beta (=0)
# rows to the output and are dead as attention sources, so each graph is
# gathered to its valid nodes (~n/2). Source/j side is padded to NJ
# (multiple of 128, partition granularity); destination/i side only to NI
# (multiple of 4, free-axis granularity). Padded slots carry x=0 and
# adj=0; the adj multiply kills them as sources and padded destination
# rows are discarded on the host scatter.
#
# Key identities used on device:
#   exp(LeakyReLU(t)) = max(exp(t), exp(0.2 t))            (t = src_i + dst_j)
#   exp(src_i + dst_j) = exp(src_i) * exp(dst_j)           (rank-1 separable)
#   adj mask applied as elementwise multiply with transposed 0/1 fp16 matrix
#   1/rowsum folds into a per-row scale after the alpha@h matmul
#   (rowsum comes free as a ones-column in the alpha@h matmul rhs).
#
# Layout: "e^T" orientation — j (softmax axis) on partitions, i on the free
# axis, so alpha@h needs no transposes and rowsum is a matmul column.
# exp-row vectors are replicated across partitions with GpSimd
# partition_broadcast; the src logit row is replicated by a PE ones-matmul
# into PSUM (ACT Prelu reads PSUM directly).
# Sharding: data-parallel, 2 graphs per core across 8 cores.

import numpy as np

H, D = 4, 64
EPS = 1e-5
NCORES = 8

_PROG_CACHE = {}


def _build_program(ng, nj, ni, in_dim, trivial_ln):
    import concourse.bacc as bacc
    import concourse.mybir as mybir
    import concourse.tile as tile
    from concourse.bass import ts

    f16 = mybir.dt.float16
    f32 = mybir.dt.float32
    AF = mybir.ActivationFunctionType
    OP = mybir.AluOpType

    HD = H * D
    NCHJ = nj // 128        # source-node chunks (partition side)
    ICW = [
        (o, min(128, ni - o)) for o in range(0, ni, 128)
    ]                       # dest-node chunks (i side, last may be short)
    KC = in_dim // 128      # contraction chunks for x@W
    # matmul moving-column chunks (each <=512, PSUM-bank aligned)
    NWCH = []
    _off = 0
    while _off < ni:
        _w = min(512, ni - _off)
        NWCH.append((_off, _w))
        _off += _w
    E = D + 1               # head block in hones (64 h cols + 1 ones col)

    nc = bacc.Bacc()

    xT = nc.dram_tensor("xT", [ng, in_dim, nj], f16, kind="ExternalInput")
    adjT = nc.dram_tensor("adjT", [ng, nj, ni], f16, kind="ExternalInput")
    wc = nc.dram_tensor("wc", [128, KC * (HD + H)], f16, kind="ExternalInput")
    wsd = nc.dram_tensor("wsd", [128, KC * H], f16, kind="ExternalInput")
    ones16 = nc.dram_tensor("ones16", [1, 128], f16, kind="ExternalInput")
    if not trivial_ln:
        gam = nc.dram_tensor("gamma_rep", [128, HD], f32, kind="ExternalInput")
        bet = nc.dram_tensor("beta_rep", [128, HD], f32, kind="ExternalInput")
    out = nc.dram_tensor("out", [ng, ni, HD], f16, kind="ExternalOutput")

    from contextlib import ExitStack

    with tile.TileContext(nc) as tc, ExitStack() as ctx:
        def pool(**kw):
            return ctx.enter_context(tc.tile_pool(**kw))

        consts = pool(name="consts", bufs=1)
        xt_pool = pool(name="xt", bufs=2 * KC)
        adjt_pool = pool(name="adjt", bufs=2 * NCHJ)
        rows_pool = pool(name="rows", bufs=2)
        reps_pool = pool(name="reps", bufs=3)
        hones_pool = pool(name="hones", bufs=NCHJ + 2)
        small_pool = pool(name="small", bufs=2)
        ew_pool = pool(name="ew", bufs=4)
        lr_pool = pool(name="lr", bufs=3)
        u_pool = pool(name="u", bufs=NCHJ + 2)
        osb_pool = pool(name="osb", bufs=2 * len(ICW))
        ln_pool = pool(name="ln", bufs=4)
        misc_pool = pool(name="misc", bufs=6)
        # PSUM pools (8 banks: ph 1x1 + pbig 2x2 + pav 3x1)
        ph_pool = pool(name="ph", bufs=2, space="PSUM")
        pbig_pool = pool(name="pbig", bufs=2, space="PSUM")
        pav_pool = pool(name="pav", bufs=2, space="PSUM")
        if True:
            # ---- constants ----
            wc_sb = consts.tile([128, KC * (HD + H)], f16, tag="wc")
            nc.sync.dma_start(wc_sb[:], wc[:])
            wsd_sb = consts.tile([128, KC * H], f16, tag="wsd")
            nc.sync.dma_start(wsd_sb[:], wsd[:])
            ones16_sb = consts.tile([1, 128], f16, tag="ones16")
            nc.sync.dma_start(ones16_sb[:], ones16[:])
            if not trivial_ln:
                gam_sb = consts.tile([128, HD], f32, tag="gam")
                nc.sync.dma_start(gam_sb[:], gam[:])
                bet_sb = consts.tile([128, HD], f32, tag="bet")
                nc.sync.dma_start(bet_sb[:], bet[:])
            eps_sb = consts.tile([128, 1], f32, tag="eps")
            nc.vector.memset(eps_sb[:], EPS)

            per_graph = []
            for g in range(ng):
                # ---- input DMAs ----
                # xT[kc]: [128, nj] fp16 (transposed on host)
                xt = []
                for kc in range(KC):
                    t = xt_pool.tile([128, nj], f16, tag="xt")
                    nc.sync.dma_start(t[:], xT[g, ts(kc, 128), :])
                    xt.append(t)
                # adjT[jc]: [128, ni] fp16 (adjT[j, i] = adj[i, j], host-side)
                adjt = []
                for jc in range(NCHJ):
                    t = adjt_pool.tile([128, ni], f16, tag="adjt")
                    nc.sync.dma_start(t[:], adjT[g, ts(jc, 128), :])
                    adjt.append(t)

                # ---- src rows: psd[h, i] = (x @ Wa_src)^T ----
                psd = pbig_pool.tile([H, ni], f32, tag="pbig")
                for off, w in NWCH:
                    for kc in range(KC):
                        nc.tensor.matmul(
                            psd[:, off : off + w],
                            wsd_sb[:, ts(kc, H)],
                            xt[kc][:, off : off + w],
                            start=(kc == 0),
                            stop=(kc == KC - 1),
                        )
                srow = rows_pool.tile([H, ni], f16, tag="srow")
                nc.scalar.copy(srow[:], psd[:])
                arow = rows_pool.tile([H, ni], f16, tag="arow")
                nc.scalar.activation(arow[:], psd[:], AF.Exp)
                crow = rows_pool.tile([H, ni], f16, tag="crow")
                nc.scalar.activation(crow[:], psd[:], AF.Exp, scale=0.2)
                # flatten to partition 0 (PE rhs / broadcast src need p0)
                srowx = rows_pool.tile([1, H * ni], f16, tag="srowx")
                nc.sync.dma_start(
                    srowx[:].rearrange("p (h w) -> p h w", h=H), srow[:]
                )
                arowx = rows_pool.tile([1, H * ni], f16, tag="arowx")
                nc.sync.dma_start(
                    arowx[:].rearrange("p (h w) -> p h w", h=H), arow[:]
                )
                crowx = rows_pool.tile([1, H * ni], f16, tag="crowx")
                nc.sync.dma_start(
                    crowx[:].rearrange("p (h w) -> p h w", h=H), crow[:]
                )

                # ---- h_ext per chunk: h (fp16, with ones col) + dst logits ----
                hones = []
                dstm = small_pool.tile([128, NCHJ * H], f32, tag="dstm")
                Bm = small_pool.tile([128, NCHJ * H], f32, tag="bm")
                Dm = small_pool.tile([128, NCHJ * H], f32, tag="dm")
                for jcc in range(NCHJ):
                    ph = ph_pool.tile([128, HD + H], f32, tag="ph")
                    for kc in range(KC):
                        nc.tensor.matmul(
                            ph[:],
                            xt[kc][:, ts(jcc, 128)],
                            wc_sb[:, ts(kc, HD + H)],
                            start=(kc == 0),
                            stop=(kc == KC - 1),
                        )
                    ho = hones_pool.tile([128, H * E], f16, tag="hones")
                    ho3 = ho[:].rearrange("p (h e) -> p h e", h=H)
                    nc.vector.tensor_copy(
                        ho3[:, :, 0:D],
                        ph[:, 0:HD].rearrange("p (h d) -> p h d", h=H),
                    )
                    nc.vector.memset(ho3[:, :, D : D + 1], 1.0)
                    hones.append(ho)
                    nc.vector.tensor_copy(
                        dstm[:, ts(jcc, H)], ph[:, HD : HD + H]
                    )
                nc.scalar.activation(Bm[:], dstm[:], AF.Exp)
                nc.scalar.activation(Dm[:], dstm[:], AF.Exp, scale=0.2)

                # ---- per head: replicate rows, elementwise, alpha@h ----
                o_sb = [
                    osb_pool.tile([128, HD], f16, tag="osb", name=f"osb_{g}_{i}")
                    for i, (ico, icw) in enumerate(ICW)
                ]
                mv8 = ln_pool.tile(
                    [128, 2 * len(ICW)], f32, tag="mv8", name=f"mv8_{g}"
                )
                if ICW[-1][1] < 128:
                    # last i-chunk is short; init stats rows the Ln pass
                    # reads but bn_aggr never writes
                    nc.vector.memset(mv8[:], 1.0)
                for h in range(H):
                    # route A (ACT Prelu+Exp, srep via PE into PSUM),
                    # route B (DVE separable max; max on GpSimd for half)
                    na = max(0, NCHJ - 2 + (1 if h < 2 else 0))
                    a_jcs = list(range(na))
                    b_jcs = list(range(na, NCHJ))

                    srep = None
                    if a_jcs:
                        srep = pbig_pool.tile([128, ni], f32, tag="pbig")
                        for off, w in NWCH:
                            nc.tensor.matmul(
                                srep[:, off : off + w],
                                ones16_sb[:],
                                srowx[0:1, h * ni + off : h * ni + off + w],
                                start=True,
                                stop=True,
                            )
                    arep = crep = None
                    if b_jcs:
                        arep = reps_pool.tile([128, ni], f16, tag="arep")
                        nc.gpsimd.partition_broadcast(
                            arep[:], arowx[0:1, h * ni : (h + 1) * ni]
                        )
                        crep = reps_pool.tile([128, ni], f16, tag="crep")
                        nc.gpsimd.partition_broadcast(
                            crep[:], crowx[0:1, h * ni : (h + 1) * ni]
                        )

                    u_tiles = [None] * NCHJ
                    for jc in a_jcs:
                        lrt = lr_pool.tile([128, ni], f16, tag="lrt")
                        nc.scalar.activation(
                            lrt[:], srep[:], AF.Prelu,
                            bias=dstm[:, jc * H + h : jc * H + h + 1],
                            alpha=0.2,
                        )
                        up = ew_pool.tile([128, ni], f16, tag="up")
                        nc.scalar.activation(up[:], lrt[:], AF.Exp)
                        u = u_pool.tile([128, ni], f16, tag="u")
                        nc.vector.tensor_mul(u[:], up[:], adjt[jc][:])
                        u_tiles[jc] = u
                    for k, jc in enumerate(b_jcs):
                        t2 = ew_pool.tile([128, ni], f16, tag="t2")
                        nc.vector.tensor_scalar(
                            t2[:], crep[:],
                            Dm[:, jc * H + h : jc * H + h + 1],
                            None, op0=OP.mult,
                        )
                        w = ew_pool.tile([128, ni], f16, tag="w")
                        nc.vector.scalar_tensor_tensor(
                            w[:], arep[:],
                            Bm[:, jc * H + h : jc * H + h + 1],
                            t2[:], op0=OP.mult, op1=OP.max,
                        )
                        u = u_pool.tile([128, ni], f16, tag="u")
                        nc.vector.tensor_mul(u[:], w[:], adjt[jc][:])
                        u_tiles[jc] = u

                    for ic, (ico, icw) in enumerate(ICW):
                        pav = pav_pool.tile([128, E], f32, tag="pav")
                        for jc in range(NCHJ):
                            nc.tensor.matmul(
                                pav[: icw, :],
                                u_tiles[jc][:, ico : ico + icw],
                                hones[jc][:, ts(h, E)],
                                start=(jc == 0),
                                stop=(jc == NCHJ - 1),
                            )
                        rs = ln_pool.tile([128, 1], f32, tag="rs")
                        nc.vector.reciprocal(rs[: icw, :], pav[: icw, D : D + 1])
                        if h < 2:
                            nc.scalar.activation(
                                o_sb[ic][: icw, ts(h, D)], pav[: icw, 0:D],
                                AF.Identity, scale=rs[: icw, :],
                            )
                        else:
                            nc.vector.tensor_scalar(
                                o_sb[ic][: icw, ts(h, D)],
                                pav[: icw, 0:D],
                                rs[: icw, :],
                                None,
                                op0=OP.mult,
                            )
                        if h == H - 1:
                            # LN stats as soon as this chunk's last head lands
                            st6 = ln_pool.tile([128, 6], f32, tag="st6")
                            nc.vector.bn_stats(st6[: icw, :], o_sb[ic][: icw, :])
                            nc.vector.bn_aggr(mv8[: icw, 2 * ic : 2 * ic + 2], st6[: icw, :])

                per_graph.append((g, o_sb, mv8))

            # ---- LayerNorm apply + output, both graphs batched at the end
            # (a single ACT-table switch to sqrt for the whole program) ----
            NIC = len(ICW)
            for g, o_sb, mv8 in per_graph:
                sd8 = ln_pool.tile([128, NIC], f32, tag="sd8")
                nc.scalar.activation(
                    sd8[:],
                    mv8[:].rearrange("p (c two) -> p c two", two=2)[:, :, 1],
                    AF.Sqrt,
                    bias=eps_sb[:],
                )
                rstd8 = ln_pool.tile([128, NIC], f32, tag="rstd8")
                nc.vector.reciprocal(rstd8[:], sd8[:])
                for ic, (ico, icw) in enumerate(ICW):
                    o2 = misc_pool.tile([128, HD], f16, tag="o2")
                    nc.vector.tensor_scalar(
                        o2[: icw, :],
                        o_sb[ic][: icw, :],
                        mv8[: icw, 2 * ic : 2 * ic + 1],
                        rstd8[: icw, ic : ic + 1],
                        op0=OP.subtract,
                        op1=OP.mult,
                    )
                    if not trivial_ln:
                        nc.vector.tensor_mul(o2[: icw, :], o2[: icw, :], gam_sb[: icw, :])
                        nc.vector.tensor_add(o2[: icw, :], o2[: icw, :], bet_sb[: icw, :])
                    hw1 = min(64, icw)
                    nc.gpsimd.dma_start(
                        out[g, ico : ico + hw1, :], o2[:hw1, :]
                    )
                    if icw > 64:
                        nc.gpsimd.dma_start(
                            out[g, ico + 64 : ico + icw, :], o2[64:icw, :]
                        )

    nc.compile()
    return nc


def _host_prep(xc, adjc, W, a_src, a_dst, gamma, beta, ng, trivial_ln):
    """Build per-core input maps (host-side folding + dtype packing only)."""
    b, in_dim, nj = xc.shape
    HD = H * D
    KC = in_dim // 128

    # Fold attention vectors into W:  Wa[c, h] = sum_d W[c, h*D+d] * a[h, d]
    Wr = W.astype(np.float64).reshape(in_dim, H, D)
    wa_src = np.einsum("chd,hd->ch", Wr, a_src.astype(np.float64))
    wa_dst = np.einsum("chd,hd->ch", Wr, a_dst.astype(np.float64))

    wc_full = np.ascontiguousarray(
        np.concatenate(
            [W.astype(np.float16), wa_dst.astype(np.float16)], axis=1
        )
        .reshape(KC, 128, HD + H)
        .transpose(1, 0, 2)
    ).reshape(128, KC * (HD + H))
    wsd_full = np.ascontiguousarray(
        wa_src.astype(np.float16).reshape(KC, 128, H).transpose(1, 0, 2)
    ).reshape(128, KC * H)
    ones16 = np.ones((1, 128), np.float16)

    in_maps = []
    for c in range(NCORES):
        gs = slice(c * ng, (c + 1) * ng)
        m = {
            "xT": xc[gs],
            "adjT": adjc[gs],
            "wc": wc_full,
            "wsd": wsd_full,
            "ones16": ones16,
        }
        if not trivial_ln:
            m["gamma_rep"] = np.ascontiguousarray(
                np.broadcast_to(gamma.astype(np.float32), (128, HD))
            )
            m["beta_rep"] = np.ascontiguousarray(
                np.broadcast_to(beta.astype(np.float32), (128, HD))
            )
        in_maps.append(m)
    return in_maps


def kernel(x, adj, mask, W, a_src, a_dst, gamma, beta, _trace=False):
    from concourse.bass_utils import run_bass_kernel_spmd

    b, n, in_dim = x.shape
    ng = b // NCORES
    HD = H * D
    trivial_ln = bool(np.all(gamma == 1.0) and np.all(beta == 0.0))

    # ---- compact each graph to its valid (mask=1) nodes ----
    maskb = np.asarray(mask) > 0
    counts = maskb.sum(1).astype(np.int64)  # valid nodes per graph
    mx = int(counts.max())
    NJ = max(128, -(-mx // 128) * 128)   # source side: partition granularity
    NI = min(NJ, max(4, -(-mx // 4) * 4))  # dest side: free-axis granularity
    idxs = [np.nonzero(maskb[g])[0] for g in range(b)]
    # host-side transposes: xT [in_dim, NJ], adjT [NJ, NI]
    xc = np.zeros((b, in_dim, NJ), np.float16)
    adjc = np.zeros((b, NJ, NI), np.float16)
    for g in range(b):
        k = int(counts[g])
        xc[g, :, :k] = x[g, idxs[g]].T
        adjc[g, :k, :k] = (adj[g][np.ix_(idxs[g], idxs[g])] != 0).T

    key = (ng, NJ, NI, in_dim, trivial_ln)
    if key not in _PROG_CACHE:
        _PROG_CACHE[key] = _build_program(*key)
    nc = _PROG_CACHE[key]

    in_maps = _host_prep(xc, adjc, W, a_src, a_dst, gamma, beta, ng, trivial_ln)
    res = run_bass_kernel_spmd(
        nc, in_maps, core_ids=list(range(NCORES)), trace=_trace
    )
    outs = [res.results[c]["out"].reshape(ng, NI, HD) for c in range(NCORES)]
    packed = np.concatenate(outs, axis=0).astype(np.float32)
    full = np.zeros((b, n, HD), np.float32)
    for g in range(b):
        full[g, idxs[g]] = packed[g, : int(counts[g])]
    if _trace:
        return full, res
    return full


# revision 31
# speedup vs baseline: 1.0277x; 1.0277x over previous
# Dense GAT layer (4 heads, dim 64) on Trainium2 via Bass/Tile.
#
# Math: h = x@W; e_ij = LeakyReLU(src_i + dst_j, 0.2); masked softmax over j
# with valid = adj & mask_i & mask_j; out = LN((alpha @ h) * mask_i).
#
# Host-side compaction: masked (mask=0) nodes contribute exactly beta (=0)
# rows to the output and are dead as attention sources, so each graph is
# gathered to its valid nodes (~n/2). Source/j side is padded to NJ
# (multiple of 128, partition granularity); destination/i side only to NI
# (multiple of 4, free-axis granularity). Padded slots carry x=0 and
# adj=0; the adj multiply kills them as sources and padded destination
# rows are discarded on the host scatter.
#
# Key identities used on device:
#   exp(LeakyReLU(t)) = max(exp(t), exp(0.2 t))            (t = src_i + dst_j)
#   exp(src_i + dst_j) = exp(src_i) * exp(dst_j)           (rank-1 separable)
#   adj mask applied as elementwise multiply with transposed 0/1 fp16 matrix
#   1/rowsum folds into a per-row scale after the alpha@h matmul
#   (rowsum comes free as a ones-column in the alpha@h matmul rhs).
#
# Layout: "e^T" orientation — j (softmax axis) on partitions, i on the free
# axis, so alpha@h needs no transposes and rowsum is a matmul column.
# exp-row vectors are replicated across partitions with GpSimd
# partition_broadcast; the src logit row is replicated by a PE ones-matmul
# into PSUM (ACT Prelu reads PSUM directly).
# Sharding: data-parallel, 2 graphs per core across 8 cores.

import numpy as np

H, D = 4, 64
EPS = 1e-5
NCORES = 8

_PROG_CACHE = {}


def _build_program(ng, nj, ni, in_dim, trivial_ln):
    import concourse.bacc as bacc
    import concourse.mybir as mybir
    import concourse.tile as tile
    from concourse.bass import ts

    f16 = mybir.dt.float16
    f32 = mybir.dt.float32
    AF = mybir.ActivationFunctionType
    OP = mybir.AluOpType

    HD = H * D
    NCHJ = nj // 128        # source-node chunks (partition side)
    ICW = [
        (o, min(128, ni - o)) for o in range(0, ni, 128)
    ]                       # dest-node chunks (i side, last may be short)
    KC = in_dim // 128      # contraction chunks for x@W
    # matmul moving-column chunks (each <=512, PSUM-bank aligned)
    NWCH = []
    _off = 0
    while _off < ni:
        _w = min(512, ni - _off)
        NWCH.append((_off, _w))
        _off += _w
    E = D + 1               # head block in hones (64 h cols + 1 ones col)

    nc = bacc.Bacc()

    xT = nc.dram_tensor("xT", [ng, in_dim, nj], f16, kind="ExternalInput")
    adjT = nc.dram_tensor("adjT", [ng, nj, ni], f16, kind="ExternalInput")
    wc = nc.dram_tensor("wc", [128, KC * (HD + H)], f16, kind="ExternalInput")
    wsd = nc.dram_tensor("wsd", [128, KC * H], f16, kind="ExternalInput")
    ones16 = nc.dram_tensor("ones16", [1, 128], f16, kind="ExternalInput")
    if not trivial_ln:
        gam = nc.dram_tensor("gamma_rep", [128, HD], f32, kind="ExternalInput")
        bet = nc.dram_tensor("beta_rep", [128, HD], f32, kind="ExternalInput")
    out = nc.dram_tensor("out", [ng, ni, HD], f16, kind="ExternalOutput")

    from contextlib import ExitStack

    with tile.TileContext(nc) as tc, ExitStack() as ctx:
        def pool(**kw):
            return ctx.enter_context(tc.tile_pool(**kw))

        consts = pool(name="consts", bufs=1)
        xt_pool = pool(name="xt", bufs=2 * KC)
        adjt_pool = pool(name="adjt", bufs=2 * NCHJ)
        rows_pool = pool(name="rows", bufs=2)
        reps_pool = pool(name="reps", bufs=3)
        hones_pool = pool(name="hones", bufs=NCHJ + 2)
        small_pool = pool(name="small", bufs=2)
        ew_pool = pool(name="ew", bufs=4)
        lr_pool = pool(name="lr", bufs=3)
        u_pool = pool(name="u", bufs=NCHJ + 2)
        osb_pool = pool(name="osb", bufs=2 * len(ICW))
        ln_pool = pool(name="ln", bufs=4)
        misc_pool = pool(name="misc", bufs=6)
        # PSUM pools (8 banks: ph 1x1 + pbig 2x2 + pav 3x1)
        ph_pool = pool(name="ph", bufs=1, space="PSUM")
        pbig_pool = pool(name="pbig", bufs=2, space="PSUM")
        pav_pool = pool(name="pav", bufs=3, space="PSUM")
        if True:
            # ---- constants ----
            wc_sb = consts.tile([128, KC * (HD + H)], f16, tag="wc")
            nc.scalar.dma_start(wc_sb[0:64, :], wc[0:64, :])
            nc.scalar.dma_start(wc_sb[64:128, :], wc[64:128, :])
            wsd_sb = consts.tile([128, KC * H], f16, tag="wsd")
            nc.scalar.dma_start(wsd_sb[:], wsd[:])
            ones16_sb = consts.tile([1, 128], f16, tag="ones16")
            nc.scalar.dma_start(ones16_sb[:], ones16[:])
            if not trivial_ln:
                gam_sb = consts.tile([128, HD], f32, tag="gam")
                nc.sync.dma_start(gam_sb[:], gam[:])
                bet_sb = consts.tile([128, HD], f32, tag="bet")
                nc.sync.dma_start(bet_sb[:], bet[:])
            eps_sb = consts.tile([128, 1], f32, tag="eps")
            nc.vector.memset(eps_sb[:], EPS)

            per_graph = []
            for g in range(ng):
                # ---- input DMAs ----
                # xT[kc]: [128, nj] fp16 (transposed on host)
                xt = []
                if g == 0:
                    xcols = []
                    o = 0
                    while o < nj:
                        w = min(256, nj - o)
                        xcols.append((o, w))
                        o += w
                    for kc in range(KC):
                        xtile = xt_pool.tile(
                            [128, nj], f16, tag="xt", name=f"xt_{g}_{kc}"
                        )
                        xt.append(xtile)
                    for o, w in xcols:
                        for kc in range(KC):
                            nc.sync.dma_start(
                                xt[kc][:, o : o + w],
                                xT[g, ts(kc, 128), o : o + w],
                            )
                else:
                    for kc in range(KC):
                        t = xt_pool.tile([128, nj], f16, tag="xt")
                        nc.sync.dma_start(t[:], xT[g, ts(kc, 128), :])
                        xt.append(t)
                # adjT[jc]: [128, ni] fp16 (adjT[j, i] = adj[i, j], host-side)
                adjt = []
                for jc in range(NCHJ):
                    t = adjt_pool.tile([128, ni], f16, tag="adjt")
                    nc.sync.dma_start(t[:], adjT[g, ts(jc, 128), :])
                    adjt.append(t)

                # ---- src rows: psd[h, i] = (x @ Wa_src)^T ----
                psd = pbig_pool.tile([H, ni], f32, tag="pbig")
                for off, w in NWCH:
                    for kc in range(KC):
                        nc.tensor.matmul(
                            psd[:, off : off + w],
                            wsd_sb[:, ts(kc, H)],
                            xt[kc][:, off : off + w],
                            start=(kc == 0),
                            stop=(kc == KC - 1),
                        )
                srow = rows_pool.tile([H, ni], f16, tag="srow")
                nc.scalar.copy(srow[:], psd[:])
                arow = rows_pool.tile([H, ni], f16, tag="arow")
                nc.scalar.activation(arow[:], psd[:], AF.Exp)
                crow = rows_pool.tile([H, ni], f16, tag="crow")
                nc.scalar.activation(crow[:], psd[:], AF.Exp, scale=0.2)
                # flatten to partition 0 (PE rhs / broadcast src need p0)
                srowx = rows_pool.tile([1, H * ni], f16, tag="srowx")
                nc.sync.dma_start(
                    srowx[:].rearrange("p (h w) -> p h w", h=H), srow[:]
                )
                arowx = rows_pool.tile([1, H * ni], f16, tag="arowx")
                nc.sync.dma_start(
                    arowx[:].rearrange("p (h w) -> p h w", h=H), arow[:]
                )
                crowx = rows_pool.tile([1, H * ni], f16, tag="crowx")
                nc.sync.dma_start(
                    crowx[:].rearrange("p (h w) -> p h w", h=H), crow[:]
                )

                # ---- h_ext per chunk: h (fp16, with ones col) + dst logits ----
                hones = []
                dstm = small_pool.tile([128, NCHJ * H], f32, tag="dstm")
                Bm = small_pool.tile([128, NCHJ * H], f32, tag="bm")
                Dm = small_pool.tile([128, NCHJ * H], f32, tag="dm")
                for jcc in range(NCHJ):
                    ph = ph_pool.tile([128, HD + H], f32, tag="ph")
                    for kc in range(KC):
                        nc.tensor.matmul(
                            ph[:],
                            xt[kc][:, ts(jcc, 128)],
                            wc_sb[:, ts(kc, HD + H)],
                            start=(kc == 0),
                            stop=(kc == KC - 1),
                        )
                    ho = hones_pool.tile([128, H * E], f16, tag="hones")
                    ho3 = ho[:].rearrange("p (h e) -> p h e", h=H)
                    nc.vector.tensor_copy(
                        ho3[:, :, 0:D],
                        ph[:, 0:HD].rearrange("p (h d) -> p h d", h=H),
                    )
                    nc.vector.memset(ho3[:, :, D : D + 1], 1.0)
                    hones.append(ho)
                    nc.vector.tensor_copy(
                        dstm[:, ts(jcc, H)], ph[:, HD : HD + H]
                    )
                nc.scalar.activation(Bm[:], dstm[:], AF.Exp)
                nc.scalar.activation(Dm[:], dstm[:], AF.Exp, scale=0.2)

                # ---- per head: replicate rows, elementwise, alpha@h ----
                o_sb = [
                    osb_pool.tile([128, HD], f16, tag="osb", name=f"osb_{g}_{i}")
                    for i, (ico, icw) in enumerate(ICW)
                ]
                mv8 = ln_pool.tile(
                    [128, 2 * len(ICW)], f32, tag="mv8", name=f"mv8_{g}"
                )
                if ICW[-1][1] < 128:
                    # last i-chunk is short; init stats rows the Ln pass
                    # reads but bn_aggr never writes
                    nc.vector.memset(mv8[:], 1.0)
                for h in range(H):
                    # route A (ACT Prelu+Exp, srep via PE into PSUM),
                    # route B (DVE separable max; max on GpSimd for half)
                    na = max(0, NCHJ - 2 + (1 if h < 2 else 0))
                    a_jcs = list(range(na))
                    b_jcs = list(range(na, NCHJ))

                    srep = None
                    if a_jcs:
                        srep = pbig_pool.tile([128, ni], f32, tag="pbig")
                        for off, w in NWCH:
                            nc.tensor.matmul(
                                srep[:, off : off + w],
                                ones16_sb[:],
                                srowx[0:1, h * ni + off : h * ni + off + w],
                                start=True,
                                stop=True,
                            )
                    arep = crep = None
                    if b_jcs:
                        arep = reps_pool.tile([128, ni], f16, tag="arep")
                        nc.gpsimd.partition_broadcast(
                            arep[:], arowx[0:1, h * ni : (h + 1) * ni]
                        )
                        crep = reps_pool.tile([128, ni], f16, tag="crep")
                        nc.gpsimd.partition_broadcast(
                            crep[:], crowx[0:1, h * ni : (h + 1) * ni]
                        )

                    u_tiles = [None] * NCHJ
                    for jc in a_jcs:
                        lrt = lr_pool.tile([128, ni], f16, tag="lrt")
                        nc.scalar.activation(
                            lrt[:], srep[:], AF.Prelu,
                            bias=dstm[:, jc * H + h : jc * H + h + 1],
                            alpha=0.2,
                        )
                        up = ew_pool.tile([128, ni], f16, tag="up")
                        nc.scalar.activation(up[:], lrt[:], AF.Exp)
                        u = u_pool.tile([128, ni], f16, tag="u")
                        nc.vector.tensor_mul(u[:], up[:], adjt[jc][:])
                        u_tiles[jc] = u
                    for k, jc in enumerate(b_jcs):
                        t2 = ew_pool.tile([128, ni], f16, tag="t2")
                        nc.vector.tensor_scalar(
                            t2[:], crep[:],
                            Dm[:, jc * H + h : jc * H + h + 1],
                            None, op0=OP.mult,
                        )
                        w = ew_pool.tile([128, ni], f16, tag="w")
                        nc.vector.scalar_tensor_tensor(
                            w[:], arep[:],
                            Bm[:, jc * H + h : jc * H + h + 1],
                            t2[:], op0=OP.mult, op1=OP.max,
                        )
                        u = u_pool.tile([128, ni], f16, tag="u")
                        nc.vector.tensor_mul(u[:], w[:], adjt[jc][:])
                        u_tiles[jc] = u

                    for ic, (ico, icw) in enumerate(ICW):
                        pav = pav_pool.tile([128, E], f32, tag="pav")
                        for jc in range(NCHJ):
                            nc.tensor.matmul(
                                pav[: icw, :],
                                u_tiles[jc][:, ico : ico + icw],
                                hones[jc][:, ts(h, E)],
                                start=(jc == 0),
                                stop=(jc == NCHJ - 1),
                            )
                        rs = ln_pool.tile([128, 1], f32, tag="rs")
                        nc.vector.reciprocal(rs[: icw, :], pav[: icw, D : D + 1])
                        if h < 2:
                            nc.scalar.activation(
                                o_sb[ic][: icw, ts(h, D)], pav[: icw, 0:D],
                                AF.Identity, scale=rs[: icw, :],
                            )
                        else:
                            nc.vector.tensor_scalar(
                                o_sb[ic][: icw, ts(h, D)],
                                pav[: icw, 0:D],
                                rs[: icw, :],
                                None,
                                op0=OP.mult,
                            )
                        if h == H - 1:
                            # LN stats as soon as this chunk's last head lands
                            st6 = ln_pool.tile([128, 6], f32, tag="st6")
                            nc.vector.bn_stats(st6[: icw, :], o_sb[ic][: icw, :])
                            nc.vector.bn_aggr(mv8[: icw, 2 * ic : 2 * ic + 2], st6[: icw, :])

                per_graph.append((g, o_sb, mv8))

            # ---- LayerNorm apply + output, both graphs batched at the end
            # (a single ACT-table switch to sqrt for the whole program) ----
            NIC = len(ICW)
            for g, o_sb, mv8 in per_graph:
                sd8 = ln_pool.tile([128, NIC], f32, tag="sd8")
                nc.scalar.activation(
                    sd8[:],
                    mv8[:].rearrange("p (c two) -> p c two", two=2)[:, :, 1],
                    AF.Sqrt,
                    bias=eps_sb[:],
                )
                rstd8 = ln_pool.tile([128, NIC], f32, tag="rstd8")
                nc.vector.reciprocal(rstd8[:], sd8[:])
                for ic, (ico, icw) in enumerate(ICW):
                    o2 = misc_pool.tile([128, HD], f16, tag="o2")
                    nc.vector.tensor_scalar(
                        o2[: icw, :],
                        o_sb[ic][: icw, :],
                        mv8[: icw, 2 * ic : 2 * ic + 1],
                        rstd8[: icw, ic : ic + 1],
                        op0=OP.subtract,
                        op1=OP.mult,
                    )
                    if not trivial_ln:
                        nc.vector.tensor_mul(o2[: icw, :], o2[: icw, :], gam_sb[: icw, :])
                        nc.vector.tensor_add(o2[: icw, :], o2[: icw, :], bet_sb[: icw, :])
                    hw1 = min(64, icw)
                    nc.gpsimd.dma_start(
                        out[g, ico : ico + hw1, :], o2[:hw1, :]
                    )
                    if icw > 64:
                        nc.gpsimd.dma_start(
                            out[g, ico + 64 : ico + icw, :], o2[64:icw, :]
                        )

    nc.compile()
    return nc


def _host_prep(xc, adjc, W, a_src, a_dst, gamma, beta, ng, trivial_ln):
    """Build per-core input maps (host-side folding + dtype packing only)."""
    b, in_dim, nj = xc.shape
    HD = H * D
    KC = in_dim // 128

    # Fold attention vectors into W:  Wa[c, h] = sum_d W[c, h*D+d] * a[h, d]
    Wr = W.astype(np.float64).reshape(in_dim, H, D)
    wa_src = np.einsum("chd,hd->ch", Wr, a_src.astype(np.float64))
    wa_dst = np.einsum("chd,hd->ch", Wr, a_dst.astype(np.float64))

    wc_full = np.ascontiguousarray(
        np.concatenate(
            [W.astype(np.float16), wa_dst.astype(np.float16)], axis=1
        )
        .reshape(KC, 128, HD + H)
        .transpose(1, 0, 2)
    ).reshape(128, KC * (HD + H))
    wsd_full = np.ascontiguousarray(
        wa_src.astype(np.float16).reshape(KC, 128, H).transpose(1, 0, 2)
    ).reshape(128, KC * H)
    ones16 = np.ones((1, 128), np.float16)

    in_maps = []
    for c in range(NCORES):
        gs = slice(c * ng, (c + 1) * ng)
        m = {
            "xT": xc[gs],
            "adjT": adjc[gs],
            "wc": wc_full,
            "wsd": wsd_full,
            "ones16": ones16,
        }
        if not trivial_ln:
            m["gamma_rep"] = np.ascontiguousarray(
                np.broadcast_to(gamma.astype(np.float32), (128, HD))
            )
            m["beta_rep"] = np.ascontiguousarray(
                np.broadcast_to(beta.astype(np.float32), (128, HD))
            )
        in_maps.append(m)
    return in_maps


def kernel(x, adj, mask, W, a_src, a_dst, gamma, beta, _trace=False):
    from concourse.bass_utils import run_bass_kernel_spmd

    b, n, in_dim = x.shape
    ng = b // NCORES
    HD = H * D
    trivial_ln = bool(np.all(gamma == 1.0) and np.all(beta == 0.0))

    # ---- compact each graph to its valid (mask=1) nodes ----
    maskb = np.asarray(mask) > 0
    counts = maskb.sum(1).astype(np.int64)  # valid nodes per graph
    mx = int(counts.max())
    NJ = max(128, -(-mx // 128) * 128)   # source side: partition granularity
    NI = min(NJ, max(4, -(-mx // 4) * 4))  # dest side: free-axis granularity
    idxs = [np.nonzero(maskb[g])[0] for g in range(b)]
    # host-side transposes: xT [in_dim, NJ], adjT [NJ, NI]
    xc = np.zeros((b, in_dim, NJ), np.float16)
    adjc = np.zeros((b, NJ, NI), np.float16)
    for g in range(b):
        k = int(counts[g])
        xc[g, :, :k] = x[g, idxs[g]].T
        adjc[g, :k, :k] = (adj[g][np.ix_(idxs[g], idxs[g])] != 0).T

    key = (ng, NJ, NI, in_dim, trivial_ln)
    if key not in _PROG_CACHE:
        _PROG_CACHE[key] = _build_program(*key)
    nc = _PROG_CACHE[key]

    in_maps = _host_prep(xc, adjc, W, a_src, a_dst, gamma, beta, ng, trivial_ln)
    res = run_bass_kernel_spmd(
        nc, in_maps, core_ids=list(range(NCORES)), trace=_trace
    )
    outs = [res.results[c]["out"].reshape(ng, NI, HD) for c in range(NCORES)]
    packed = np.concatenate(outs, axis=0).astype(np.float32)
    full = np.zeros((b, n, HD), np.float32)
    for g in range(b):
        full[g, idxs[g]] = packed[g, : int(counts[g])]
    if _trace:
        return full, res
    return full


# revision 33
# speedup vs baseline: 1.0649x; 1.0361x over previous
# Dense GAT layer (4 heads, dim 64) on Trainium2 via Bass/Tile.
#
# Math: h = x@W; e_ij = LeakyReLU(src_i + dst_j, 0.2); masked softmax over j
# with valid = adj & mask_i & mask_j; out = LN((alpha @ h) * mask_i).
#
# Host-side compaction: masked (mask=0) nodes contribute exactly beta (=0)
# rows to the output and are dead as attention sources, so each graph is
# gathered to its valid nodes (~n/2). Source/j side is padded to NJ
# (multiple of 128, partition granularity); destination/i side only to NI
# (multiple of 4, free-axis granularity). Padded slots carry x=0 and
# adj=0; the adj multiply kills them as sources and padded destination
# rows are discarded on the host scatter.
#
# Key identities used on device:
#   exp(LeakyReLU(t)) = max(exp(t), exp(0.2 t))            (t = src_i + dst_j)
#   exp(src_i + dst_j) = exp(src_i) * exp(dst_j)           (rank-1 separable)
#   adj mask applied as elementwise multiply with transposed 0/1 fp16 matrix
#   1/rowsum folds into a per-row scale after the alpha@h matmul
#   (rowsum comes free as a ones-column in the alpha@h matmul rhs).
#
# Layout: "e^T" orientation — j (softmax axis) on partitions, i on the free
# axis, so alpha@h needs no transposes and rowsum is a matmul column.
# exp-row vectors are replicated across partitions with GpSimd
# partition_broadcast; the src logit row is replicated by a PE ones-matmul
# into PSUM (ACT Prelu reads PSUM directly).
# Sharding: data-parallel, 2 graphs per core across 8 cores.

import numpy as np

H, D = 4, 64
EPS = 1e-5
NCORES = 8

_PROG_CACHE = {}


def _build_program(ng, nj, ni, in_dim, trivial_ln):
    import concourse.bacc as bacc
    import concourse.mybir as mybir
    import concourse.tile as tile
    from concourse.bass import ts

    f16 = mybir.dt.float16
    f32 = mybir.dt.float32
    AF = mybir.ActivationFunctionType
    OP = mybir.AluOpType

    HD = H * D
    NCHJ = nj // 128        # source-node chunks (partition side)
    ICW = [
        (o, min(128, ni - o)) for o in range(0, ni, 128)
    ]                       # dest-node chunks (i side, last may be short)
    KC = in_dim // 128      # contraction chunks for x@W
    # matmul moving-column chunks (each <=512, PSUM-bank aligned)
    NWCH = []
    _off = 0
    while _off < ni:
        _w = min(512, ni - _off)
        NWCH.append((_off, _w))
        _off += _w
    E = D + 1               # head block in hones (64 h cols + 1 ones col)

    nc = bacc.Bacc()

    xT = nc.dram_tensor("xT", [ng, in_dim, nj], f16, kind="ExternalInput")
    adjT = nc.dram_tensor("adjT", [ng, nj, ni], f16, kind="ExternalInput")
    wc = nc.dram_tensor("wc", [128, KC * (HD + H)], f16, kind="ExternalInput")
    wsd = nc.dram_tensor("wsd", [128, KC * H], f16, kind="ExternalInput")
    ones16 = nc.dram_tensor("ones16", [1, 128], f16, kind="ExternalInput")
    if not trivial_ln:
        gam = nc.dram_tensor("gamma_rep", [128, HD], f32, kind="ExternalInput")
        bet = nc.dram_tensor("beta_rep", [128, HD], f32, kind="ExternalInput")
    out = nc.dram_tensor("out", [ng, ni, HD], f16, kind="ExternalOutput")

    from contextlib import ExitStack

    with tile.TileContext(nc) as tc, ExitStack() as ctx:
        def pool(**kw):
            return ctx.enter_context(tc.tile_pool(**kw))

        consts = pool(name="consts", bufs=1)
        xt_pool = pool(name="xt", bufs=2 * KC)
        adjt_pool = pool(name="adjt", bufs=2 * NCHJ)
        rows_pool = pool(name="rows", bufs=2)
        reps_pool = pool(name="reps", bufs=3)
        hones_pool = pool(name="hones", bufs=NCHJ + 2)
        small_pool = pool(name="small", bufs=2)
        ew_pool = pool(name="ew", bufs=4)
        lr_pool = pool(name="lr", bufs=3)
        u_pool = pool(name="u", bufs=NCHJ + 2)
        osb_pool = pool(name="osb", bufs=2 * len(ICW))
        ln_pool = pool(name="ln", bufs=4)
        misc_pool = pool(name="misc", bufs=6)
        # PSUM pools (8 banks: ph 1x1 + pbig 2x2 + pav 3x1)
        ph_pool = pool(name="ph", bufs=1, space="PSUM")
        pbig_pool = pool(name="pbig", bufs=2, space="PSUM")
        pav_pool = pool(name="pav", bufs=3, space="PSUM")
        if True:
            # ---- constants ----
            wc_sb = consts.tile([128, KC * (HD + H)], f16, tag="wc")
            nc.scalar.dma_start(wc_sb[0:64, :], wc[0:64, :])
            nc.scalar.dma_start(wc_sb[64:128, :], wc[64:128, :])
            wsd_sb = consts.tile([128, KC * H], f16, tag="wsd")
            nc.scalar.dma_start(wsd_sb[:], wsd[:])
            ones16_sb = consts.tile([1, 128], f16, tag="ones16")
            nc.scalar.dma_start(ones16_sb[:], ones16[:])
            if not trivial_ln:
                gam_sb = consts.tile([128, HD], f32, tag="gam")
                nc.sync.dma_start(gam_sb[:], gam[:])
                bet_sb = consts.tile([128, HD], f32, tag="bet")
                nc.sync.dma_start(bet_sb[:], bet[:])
            eps_sb = consts.tile([128, 1], f32, tag="eps")
            nc.vector.memset(eps_sb[:], EPS)

            per_graph = []
            for g in range(ng):
                # ---- input DMAs ----
                # xT[kc]: [128, nj] fp16 (transposed on host)
                xt = []
                if g == 0:
                    xcols = []
                    o = 0
                    while o < nj:
                        w = min(256, nj - o)
                        xcols.append((o, w))
                        o += w
                    for kc in range(KC):
                        xtile = xt_pool.tile(
                            [128, nj], f16, tag="xt", name=f"xt_{g}_{kc}"
                        )
                        xt.append(xtile)
                    for o, w in xcols:
                        for kc in range(KC):
                            nc.sync.dma_start(
                                xt[kc][:, o : o + w],
                                xT[g, ts(kc, 128), o : o + w],
                            )
                else:
                    for kc in range(KC):
                        t = xt_pool.tile([128, nj], f16, tag="xt")
                        nc.sync.dma_start(t[:], xT[g, ts(kc, 128), :])
                        xt.append(t)
                # adjT[jc]: [128, ni] fp16 (adjT[j, i] = adj[i, j], host-side)
                adjt = []
                for jc in range(NCHJ):
                    t = adjt_pool.tile([128, ni], f16, tag="adjt")
                    nc.sync.dma_start(t[:], adjT[g, ts(jc, 128), :])
                    adjt.append(t)

                # ---- src rows: psd[h, i] = (x @ Wa_src)^T ----
                psd = pbig_pool.tile([H, ni], f32, tag="pbig")
                for off, w in NWCH:
                    for kc in range(KC):
                        nc.tensor.matmul(
                            psd[:, off : off + w],
                            wsd_sb[:, ts(kc, H)],
                            xt[kc][:, off : off + w],
                            start=(kc == 0),
                            stop=(kc == KC - 1),
                        )
                arow = rows_pool.tile([H, ni], f16, tag="arow")
                nc.scalar.activation(arow[:], psd[:], AF.Exp)
                crow = rows_pool.tile([H, ni], f16, tag="crow")
                nc.scalar.activation(crow[:], psd[:], AF.Exp, scale=0.2)
                srow = rows_pool.tile([H, ni], f16, tag="srow")
                nc.scalar.copy(srow[:], psd[:])
                # flatten to partition 0 (PE rhs / broadcast src need p0)
                srowx = rows_pool.tile([1, H * ni], f16, tag="srowx")
                nc.sync.dma_start(
                    srowx[:].rearrange("p (h w) -> p h w", h=H), srow[:]
                )
                arowx = rows_pool.tile([1, H * ni], f16, tag="arowx")
                nc.sync.dma_start(
                    arowx[:].rearrange("p (h w) -> p h w", h=H), arow[:]
                )
                crowx = rows_pool.tile([1, H * ni], f16, tag="crowx")
                nc.sync.dma_start(
                    crowx[:].rearrange("p (h w) -> p h w", h=H), crow[:]
                )

                # ---- h_ext per chunk: h (fp16, with ones col) + dst logits ----
                hones = []
                dstm = small_pool.tile([128, NCHJ * H], f32, tag="dstm")
                Bm = small_pool.tile([128, NCHJ * H], f32, tag="bm")
                Dm = small_pool.tile([128, NCHJ * H], f32, tag="dm")
                for jcc in range(NCHJ):
                    ph = ph_pool.tile([128, HD + H], f32, tag="ph")
                    for kc in range(KC):
                        nc.tensor.matmul(
                            ph[:],
                            xt[kc][:, ts(jcc, 128)],
                            wc_sb[:, ts(kc, HD + H)],
                            start=(kc == 0),
                            stop=(kc == KC - 1),
                        )
                    ho = hones_pool.tile([128, H * E], f16, tag="hones")
                    ho3 = ho[:].rearrange("p (h e) -> p h e", h=H)
                    nc.vector.tensor_copy(
                        ho3[:, :, 0:D],
                        ph[:, 0:HD].rearrange("p (h d) -> p h d", h=H),
                    )
                    nc.vector.memset(ho3[:, :, D : D + 1], 1.0)
                    hones.append(ho)
                    nc.vector.tensor_copy(
                        dstm[:, ts(jcc, H)], ph[:, HD : HD + H]
                    )
                nc.scalar.activation(Bm[:], dstm[:], AF.Exp)
                nc.scalar.activation(Dm[:], dstm[:], AF.Exp, scale=0.2)

                # ---- per head: replicate rows, elementwise, alpha@h ----
                o_sb = [
                    osb_pool.tile([128, HD], f16, tag="osb", name=f"osb_{g}_{i}")
                    for i, (ico, icw) in enumerate(ICW)
                ]
                mv8 = ln_pool.tile(
                    [128, 2 * len(ICW)], f32, tag="mv8", name=f"mv8_{g}"
                )
                if ICW[-1][1] < 128:
                    # last i-chunk is short; init stats rows the Ln pass
                    # reads but bn_aggr never writes
                    nc.vector.memset(mv8[:], 1.0)
                for h in range(H):
                    # route A (ACT Prelu+Exp, srep via PE into PSUM),
                    # route B (DVE separable max; max on GpSimd for half)
                    na = max(0, NCHJ - 2 + (1 if h < 2 else 0))
                    a_jcs = list(range(na))
                    b_jcs = list(range(na, NCHJ))

                    # head 0's rows already sit at partition 0 — read them
                    # directly so the flatten DMA latency stays off the
                    # critical path (it only feeds heads 1..3)
                    if h == 0:
                        s_src = srow[0:1, :]
                        a_src = arow[0:1, :]
                        c_src = crow[0:1, :]
                    else:
                        s_src = srowx[0:1, h * ni : (h + 1) * ni]
                        a_src = arowx[0:1, h * ni : (h + 1) * ni]
                        c_src = crowx[0:1, h * ni : (h + 1) * ni]
                    srep = None
                    if a_jcs:
                        srep = pbig_pool.tile([128, ni], f32, tag="pbig")
                        for off, w in NWCH:
                            nc.tensor.matmul(
                                srep[:, off : off + w],
                                ones16_sb[:],
                                s_src[:, off : off + w],
                                start=True,
                                stop=True,
                            )
                    arep = crep = None
                    if b_jcs:
                        arep = reps_pool.tile([128, ni], f16, tag="arep")
                        nc.gpsimd.partition_broadcast(arep[:], a_src)
                        crep = reps_pool.tile([128, ni], f16, tag="crep")
                        nc.gpsimd.partition_broadcast(crep[:], c_src)

                    u_tiles = [None] * NCHJ
                    for jc in a_jcs:
                        lrt = lr_pool.tile([128, ni], f16, tag="lrt")
                        nc.scalar.activation(
                            lrt[:], srep[:], AF.Prelu,
                            bias=dstm[:, jc * H + h : jc * H + h + 1],
                            alpha=0.2,
                        )
                        up = ew_pool.tile([128, ni], f16, tag="up")
                        nc.scalar.activation(up[:], lrt[:], AF.Exp)
                        u = u_pool.tile([128, ni], f16, tag="u")
                        nc.vector.tensor_mul(u[:], up[:], adjt[jc][:])
                        u_tiles[jc] = u
                    for k, jc in enumerate(b_jcs):
                        t2 = ew_pool.tile([128, ni], f16, tag="t2")
                        nc.vector.tensor_scalar(
                            t2[:], crep[:],
                            Dm[:, jc * H + h : jc * H + h + 1],
                            None, op0=OP.mult,
                        )
                        w = ew_pool.tile([128, ni], f16, tag="w")
                        nc.vector.scalar_tensor_tensor(
                            w[:], arep[:],
                            Bm[:, jc * H + h : jc * H + h + 1],
                            t2[:], op0=OP.mult, op1=OP.max,
                        )
                        u = u_pool.tile([128, ni], f16, tag="u")
                        nc.vector.tensor_mul(u[:], w[:], adjt[jc][:])
                        u_tiles[jc] = u

                    for ic, (ico, icw) in enumerate(ICW):
                        pav = pav_pool.tile([128, E], f32, tag="pav")
                        for jc in range(NCHJ):
                            nc.tensor.matmul(
                                pav[: icw, :],
                                u_tiles[jc][:, ico : ico + icw],
                                hones[jc][:, ts(h, E)],
                                start=(jc == 0),
                                stop=(jc == NCHJ - 1),
                            )
                        rs = ln_pool.tile([128, 1], f32, tag="rs")
                        nc.vector.reciprocal(rs[: icw, :], pav[: icw, D : D + 1])
                        if h < 2:
                            nc.scalar.activation(
                                o_sb[ic][: icw, ts(h, D)], pav[: icw, 0:D],
                                AF.Identity, scale=rs[: icw, :],
                            )
                        else:
                            nc.vector.tensor_scalar(
                                o_sb[ic][: icw, ts(h, D)],
                                pav[: icw, 0:D],
                                rs[: icw, :],
                                None,
                                op0=OP.mult,
                            )
                        if h == H - 1:
                            # LN stats as soon as this chunk's last head lands
                            st6 = ln_pool.tile([128, 6], f32, tag="st6")
                            nc.vector.bn_stats(st6[: icw, :], o_sb[ic][: icw, :])
                            nc.vector.bn_aggr(mv8[: icw, 2 * ic : 2 * ic + 2], st6[: icw, :])

                per_graph.append((g, o_sb, mv8))

            # ---- LayerNorm apply + output, both graphs batched at the end
            # (a single ACT-table switch to sqrt for the whole program) ----
            NIC = len(ICW)
            for g, o_sb, mv8 in per_graph:
                sd8 = ln_pool.tile([128, NIC], f32, tag="sd8")
                nc.scalar.activation(
                    sd8[:],
                    mv8[:].rearrange("p (c two) -> p c two", two=2)[:, :, 1],
                    AF.Sqrt,
                    bias=eps_sb[:],
                )
                rstd8 = ln_pool.tile([128, NIC], f32, tag="rstd8")
                nc.vector.reciprocal(rstd8[:], sd8[:])
                for ic, (ico, icw) in enumerate(ICW):
                    o2 = misc_pool.tile([128, HD], f16, tag="o2")
                    nc.vector.tensor_scalar(
                        o2[: icw, :],
                        o_sb[ic][: icw, :],
                        mv8[: icw, 2 * ic : 2 * ic + 1],
                        rstd8[: icw, ic : ic + 1],
                        op0=OP.subtract,
                        op1=OP.mult,
                    )
                    if not trivial_ln:
                        nc.vector.tensor_mul(o2[: icw, :], o2[: icw, :], gam_sb[: icw, :])
                        nc.vector.tensor_add(o2[: icw, :], o2[: icw, :], bet_sb[: icw, :])
                    hw1 = min(64, icw)
                    nc.gpsimd.dma_start(
                        out[g, ico : ico + hw1, :], o2[:hw1, :]
                    )
                    if icw > 64:
                        nc.gpsimd.dma_start(
                            out[g, ico + 64 : ico + icw, :], o2[64:icw, :]
                        )

    nc.compile()
    return nc


def _host_prep(xc, adjc, W, a_src, a_dst, gamma, beta, ng, trivial_ln):
    """Build per-core input maps (host-side folding + dtype packing only)."""
    b, in_dim, nj = xc.shape
    HD = H * D
    KC = in_dim // 128

    # Fold attention vectors into W:  Wa[c, h] = sum_d W[c, h*D+d] * a[h, d]
    Wr = W.astype(np.float64).reshape(in_dim, H, D)
    wa_src = np.einsum("chd,hd->ch", Wr, a_src.astype(np.float64))
    wa_dst = np.einsum("chd,hd->ch", Wr, a_dst.astype(np.float64))

    wc_full = np.ascontiguousarray(
        np.concatenate(
            [W.astype(np.float16), wa_dst.astype(np.float16)], axis=1
        )
        .reshape(KC, 128, HD + H)
        .transpose(1, 0, 2)
    ).reshape(128, KC * (HD + H))
    wsd_full = np.ascontiguousarray(
        wa_src.astype(np.float16).reshape(KC, 128, H).transpose(1, 0, 2)
    ).reshape(128, KC * H)
    ones16 = np.ones((1, 128), np.float16)

    in_maps = []
    for c in range(NCORES):
        gs = slice(c * ng, (c + 1) * ng)
        m = {
            "xT": xc[gs],
            "adjT": adjc[gs],
            "wc": wc_full,
            "wsd": wsd_full,
            "ones16": ones16,
        }
        if not trivial_ln:
            m["gamma_rep"] = np.ascontiguousarray(
                np.broadcast_to(gamma.astype(np.float32), (128, HD))
            )
            m["beta_rep"] = np.ascontiguousarray(
                np.broadcast_to(beta.astype(np.float32), (128, HD))
            )
        in_maps.append(m)
    return in_maps


def kernel(x, adj, mask, W, a_src, a_dst, gamma, beta, _trace=False):
    from concourse.bass_utils import run_bass_kernel_spmd

    b, n, in_dim = x.shape
    ng = b // NCORES
    HD = H * D
    trivial_ln = bool(np.all(gamma == 1.0) and np.all(beta == 0.0))

    # ---- compact each graph to its valid (mask=1) nodes ----
    maskb = np.asarray(mask) > 0
    counts = maskb.sum(1).astype(np.int64)  # valid nodes per graph
    mx = int(counts.max())
    NJ = max(128, -(-mx // 128) * 128)   # source side: partition granularity
    NI = min(NJ, max(4, -(-mx // 4) * 4))  # dest side: free-axis granularity
    idxs = [np.nonzero(maskb[g])[0] for g in range(b)]
    # host-side transposes: xT [in_dim, NJ], adjT [NJ, NI]
    xc = np.zeros((b, in_dim, NJ), np.float16)
    adjc = np.zeros((b, NJ, NI), np.float16)
    for g in range(b):
        k = int(counts[g])
        xc[g, :, :k] = x[g, idxs[g]].T
        adjc[g, :k, :k] = (adj[g][np.ix_(idxs[g], idxs[g])] != 0).T

    key = (ng, NJ, NI, in_dim, trivial_ln)
    if key not in _PROG_CACHE:
        _PROG_CACHE[key] = _build_program(*key)
    nc = _PROG_CACHE[key]

    in_maps = _host_prep(xc, adjc, W, a_src, a_dst, gamma, beta, ng, trivial_ln)
    res = run_bass_kernel_spmd(
        nc, in_maps, core_ids=list(range(NCORES)), trace=_trace
    )
    outs = [res.results[c]["out"].reshape(ng, NI, HD) for c in range(NCORES)]
    packed = np.concatenate(outs, axis=0).astype(np.float32)
    full = np.zeros((b, n, HD), np.float32)
    for g in range(b):
        full[g, idxs[g]] = packed[g, : int(counts[g])]
    if _trace:
        return full, res
    return full
